# revision 2
# baseline (speedup 1.0000x reference)
"""GRU policy kernel for Trainium2 (8 NeuronCores, data-parallel over batch).

Problem: nn_GRUPolicy — B=2048, T=512, V=4, E=64, H=128.

  xe = emb[x]                          # [B,T,E]
  gi = xe @ W_ih.T + b_ih              # [B,T,3H]
  scan over t: GRU cell (PyTorch gate order r,z,n)
  logits = h_T @ W_fc.T + b_fc         # [B,V]

Key facts exploited:
  * V=4 so the input-side projection collapses into a [4, 3H] lookup
    table giTab = emb @ W_ih.T + b_ih (+ b_hh r/z folded in); per step it
    is realized on-device as a K=4 one-hot matmul accumulated straight
    into the same PSUM region as the recurrence matmul.
  * The one-hot itself is built ON DEVICE from uint8 token indices
    (1 byte/token over the slow axon link instead of 8 f16 one-hot
    bytes): a broadcast DMA replicates the index row across V=4
    partitions, then one DVE is_equal against a per-partition iota
    column yields the [V, chunk*BS] one-hot, split into pieces that
    interleave with the previous chunk's recurrence steps so the DVE
    never stalls the chain.
  * Everything is kept transposed ([H, batch] on 128 partitions) so the
    recurrence never needs a transpose.
  * h' = (1-z)*n + z*h  ==  z*h - (z-1)*n  -> one GPSIMD mult (p=z*h,
    off the critical path), one fused scalar_tensor_tensor
    (q=(z-1)*n) and one subtract.
  * b_hh_n rides for free inside the fused u = (ghn + b_hh_n) * r.

Sharding: batch 2048 -> 8 cores x 256; each core runs 2 independent
128-column chains, emitted interleaved by op-kind, so the serial
per-step dependency chain of one chain overlaps with engine work of
the other.

Host side: the jitted shard_map executable is compiled ONCE per process
and cached — repeated kernel() calls only pay input upload + execute.
"""

import sys

import numpy as np

for _p in ("/opt/trn_rl_repo",):
    if _p not in sys.path:
        sys.path.insert(0, _p)

from concourse import bacc, bass, mybir, tile  # noqa: E402

F16 = mybir.dt.float16
F32 = mybir.dt.float32
U8 = mybir.dt.uint8
I32 = mybir.dt.int32
AF = mybir.ActivationFunctionType
OP = mybir.AluOpType

B, T, V, E, H = 2048, 512, 4, 64, 128
N_CORES = 8
BS = B // N_CORES          # 256 batch rows per core
NCH = 2                    # independent chains per core
USE_GPS = True             # p = z*h on GPSIMD
WBUFS = 3                  # work pool depth
W = BS // NCH              # 128 batch columns per chain
CHUNK = 64                 # time steps per one-hot chunk
CMP_SPLIT = 16             # one-hot compare pieces per chunk


def build_nc(t_steps: int = T, dump_h: bool = False, reps: int = 1,
             nch: int = NCH, use_gps: bool = USE_GPS,
             wbufs: int = WBUFS) -> bass.Bass:
    nc = bacc.Bacc(None)

    xi_d = nc.dram_tensor("xi", [1, t_steps * BS], U8, kind="ExternalInput")
    wt_d = nc.dram_tensor("WT", [H, 3 * H], F16, kind="ExternalInput")
    gi_d = nc.dram_tensor("giT", [V, 3 * H], F16, kind="ExternalInput")
    wf_d = nc.dram_tensor("WfcT", [H, V], F16, kind="ExternalInput")
    bf_d = nc.dram_tensor("bfc", [V, 1], F32, kind="ExternalInput")
    bhn_d = nc.dram_tensor("bhn", [H, 1], F32, kind="ExternalInput")
    lo_d = nc.dram_tensor("loT", [V, BS], F32, kind="ExternalOutput")
    h_d = (
        nc.dram_tensor("hT", [H, BS], F32, kind="ExternalOutput")
        if dump_h
        else None
    )

    W = BS // nch
    chunk = min(CHUNK, t_steps)
    n_chunks = max(1, t_steps // chunk)
    ccols = chunk * BS                 # one-hot columns per chunk
    pieces = min(CMP_SPLIT, chunk)     # compare pieces per chunk
    pcols = ccols // pieces
    slot = chunk // pieces             # steps between compare pieces

    with tile.TileContext(nc) as tc:
        with (
            tc.tile_pool(name="const", bufs=1) as constp,
            tc.tile_pool(name="state", bufs=1) as statep,
            tc.tile_pool(name="xbp", bufs=2) as xbp,
            tc.tile_pool(name="ohp", bufs=2) as ohp,
            tc.tile_pool(name="work", bufs=wbufs) as workp,
            tc.tile_pool(name="psAB", bufs=2, space="PSUM") as psab,
            tc.tile_pool(name="psNG", bufs=2, space="PSUM") as psng,
        ):
            wt = constp.tile([H, 3 * H], F16, tag="wt")
            nc.sync.dma_start(wt[:], wt_d[:])
            gi = constp.tile([V, 3 * H], F16, tag="gi")
            nc.sync.dma_start(gi[:], gi_d[:])
            wf = constp.tile([H, V], F16, tag="wf")
            nc.sync.dma_start(wf[:], wf_d[:])
            bf = constp.tile([V, 1], F32, tag="bf")
            nc.sync.dma_start(bf[:], bf_d[:])
            bhn = constp.tile([H, 1], F32, tag="bhn")
            nc.sync.dma_start(bhn[:], bhn_d[:])
            lo = constp.tile([V, BS], F32, tag="lo")

            iv32 = constp.tile([V, 1], I32, tag="iv32")
            nc.gpsimd.iota(iv32[:], pattern=[[0, 1]], base=0, channel_multiplier=1)
            iv = constp.tile([V, 1], F32, tag="iv")
            nc.vector.tensor_copy(iv[:], iv32[:])

            h = []
            for c in range(nch):
                hc = statep.tile([H, W], F16, tag=f"h{c}")
                nc.vector.memset(hc[:], 0.0)
                h.append(hc)

            # flat chunk schedule across reps; one-hot build for element
            # i+1 is emitted interleaved with the steps of element i.
            seq = [ck for _rep in range(reps) for ck in range(n_chunks)]

            def emit_xb(ck):
                xb = xbp.tile([V, ccols], U8, tag="xb")
                nc.sync.dma_start(
                    xb[:],
                    xi_d[:, ck * ccols : (ck + 1) * ccols].broadcast_to([V, ccols]),
                )
                return xb

            def emit_cmp(oh_t, xb, piece):
                sl = slice(piece * pcols, (piece + 1) * pcols)
                nc.vector.tensor_scalar(
                    oh_t[:, sl], xb[:, sl], iv[:, 0:1], None, op0=OP.is_equal
                )

            # prologue: chunk 0 fully prepared
            xb_cur = emit_xb(seq[0])
            oh_cur = ohp.tile([V, ccols], F16, tag="oh")
            for pc in range(pieces):
                emit_cmp(oh_cur, xb_cur, pc)

            for i, ck in enumerate(seq):
                oh_t = oh_cur
                xb_nxt = oh_nxt = None
                for tl in range(chunk):
                    # pipelined one-hot build for the next chunk
                    if i + 1 < len(seq):
                        if tl == 0:
                            xb_nxt = emit_xb(seq[i + 1])
                            oh_nxt = ohp.tile([V, ccols], F16, tag="oh")
                        if tl % slot == slot - 1:
                            emit_cmp(oh_nxt, xb_nxt, tl // slot)

                    abs_, ngs, rzs, us, ns_, ps = {}, {}, {}, {}, {}, {}
                    for c in range(nch):
                        ohs = oh_t[:, tl * BS + c * W : tl * BS + (c + 1) * W]
                        ab = psab.tile([H, 2 * W], F32, tag=f"ab{c}", name=f"ab{c}")
                        ng = psng.tile([H, 3 * W], F32, tag=f"ng{c}", name=f"ng{c}")
                        abs_[c], ngs[c] = ab, ng

                        # a = gi_r(x_t) + W_r h   (both biases folded into gi)
                        nc.tensor.matmul(
                            ab[:, 0:W], gi[:, 0:H], ohs, start=True, stop=False
                        )
                        nc.tensor.matmul(
                            ab[:, 0:W], wt[:, 0:H], h[c][:], start=False, stop=True
                        )
                        # b = gi_z(x_t) + W_z h
                        nc.tensor.matmul(
                            ab[:, W : 2 * W],
                            gi[:, H : 2 * H],
                            ohs,
                            start=True,
                            stop=False,
                        )
                        nc.tensor.matmul(
                            ab[:, W : 2 * W],
                            wt[:, H : 2 * H],
                            h[c][:],
                            start=False,
                            stop=True,
                        )
                        # ghn = W_n h ; gin = gi_n(x_t)   (kept separate)
                        nc.tensor.matmul(
                            ng[:, 0:W], wt[:, 2 * H : 3 * H], h[c][:],
                            start=True, stop=True,
                        )
                        nc.tensor.matmul(
                            ng[:, W : 2 * W], gi[:, 2 * H : 3 * H], ohs,
                            start=True, stop=True,
                        )

                    for c in range(nch):
                        # r|z = sigmoid(a|b) in one ACT op
                        rz = workp.tile([H, 2 * W], F16, tag=f"rz{c}", name=f"rz{c}")
                        nc.scalar.activation(rz[:], abs_[c][:], AF.Sigmoid)
                        rzs[c] = rz
                    for c in range(nch):
                        # u = r * (ghn + b_hh_n) ; n-input c = u + gin (PSUM)
                        u = workp.tile([H, W], F16, tag=f"u{c}", name=f"u{c}")
                        nc.vector.scalar_tensor_tensor(
                            u[:], ngs[c][:, 0:W], bhn[:], rzs[c][:, 0:W],
                            op0=OP.add, op1=OP.mult,
                        )
                        us[c] = u
                        # p = z*h off the critical path
                        p_t = workp.tile([H, W], F16, tag=f"p{c}", name=f"p{c}")
                        peng = nc.gpsimd if use_gps else nc.vector
                        peng.tensor_mul(p_t[:], rzs[c][:, W : 2 * W], h[c][:])
                        ps[c] = p_t
                    for c in range(nch):
                        nc.vector.tensor_add(
                            ngs[c][:, 2 * W : 3 * W], us[c][:], ngs[c][:, W : 2 * W]
                        )
                    for c in range(nch):
                        n_t = workp.tile([H, W], F16, tag=f"n{c}", name=f"n{c}")
                        nc.scalar.activation(n_t[:], ngs[c][:, 2 * W : 3 * W], AF.Tanh)
                        ns_[c] = n_t
                    for c in range(nch):
                        q_t = workp.tile([H, W], F16, tag=f"q{c}", name=f"q{c}")
                        nc.vector.scalar_tensor_tensor(
                            q_t[:], rzs[c][:, W : 2 * W], 1.0, ns_[c][:],
                            op0=OP.subtract, op1=OP.mult,
                        )
                        nc.vector.tensor_sub(h[c][:], ps[c][:], q_t[:])

                xb_cur, oh_cur = xb_nxt, oh_nxt

            if h_d is not None:
                hd = constp.tile([H, BS], F32, tag="hd")
                for c in range(nch):
                    nc.vector.tensor_copy(hd[:, c * W : (c + 1) * W], h[c][:])
                nc.sync.dma_start(h_d[:], hd[:])

            # logits.T = W_fc @ h + b_fc
            for c in range(nch):
                lg = psab.tile([V, W], F32, tag="ab0")
                nc.tensor.matmul(lg[:], wf[:], h[c][:], start=True, stop=True)
                nc.scalar.activation(
                    lo[:, c * W : (c + 1) * W], lg[:], AF.Identity, bias=bf[:]
                )
            nc.sync.dma_start(lo_d[:], lo[:])

    nc.finalize()
    return nc


_NC_CACHE: dict[tuple, bass.Bass] = {}


def get_nc(t_steps: int = T, reps: int = 1, nch: int = NCH,
           use_gps: bool = USE_GPS, wbufs: int = WBUFS) -> bass.Bass:
    key = (t_steps, reps, nch, use_gps, wbufs)
    if key not in _NC_CACHE:
        _NC_CACHE[key] = build_nc(t_steps, reps=reps, nch=nch,
                                  use_gps=use_gps, wbufs=wbufs)
    return _NC_CACHE[key]


def make_in_maps(x, emb, W_ih, W_hh, b_ih, b_hh, W_fc, b_fc, t_steps: int = T):
    x = np.asarray(x)
    emb = np.asarray(emb, dtype=np.float32)
    W_ih = np.asarray(W_ih, dtype=np.float32)
    W_hh = np.asarray(W_hh, dtype=np.float32)
    b_ih = np.asarray(b_ih, dtype=np.float32)
    b_hh = np.asarray(b_hh, dtype=np.float32)
    W_fc = np.asarray(W_fc, dtype=np.float32)
    b_fc = np.asarray(b_fc, dtype=np.float32)

    # Fold b_ih (all gates) + b_hh (r,z only) into the gi lookup table.
    # b_hh_n must stay inside the reset product: n = tanh(gi_n + r*(W_n h + b_hh_n))
    bias = b_ih.copy()
    bias[: 2 * H] += b_hh[: 2 * H]
    gi_tab = (emb @ W_ih.T + bias).astype(np.float16)  # [V, 3H]
    wt = np.ascontiguousarray(W_hh.T).astype(np.float16)      # [H, 3H]
    wfc = np.ascontiguousarray(W_fc.T).astype(np.float16)     # [H, V]
    bfc = b_fc.reshape(V, 1).astype(np.float32)
    bhn = b_hh[2 * H :].reshape(H, 1).astype(np.float32)

    xiT = np.ascontiguousarray(x[:, :t_steps].T.astype(np.uint8))  # [t, B]
    in_maps = []
    for c in range(N_CORES):
        xi = xiT[:, c * BS : (c + 1) * BS].reshape(1, t_steps * BS)
        in_maps.append(
            {
                "xi": np.ascontiguousarray(xi),
                "WT": wt,
                "giT": gi_tab,
                "WfcT": wfc,
                "bfc": bfc,
                "bhn": bhn,
            }
        )
    return in_maps


def make_runner(nc, n_cores: int = N_CORES):
    """Compile nc into a reusable jitted shard_map executable.

    Mirrors concourse.bass2jax.run_bass_via_pjrt's multi-core path, but
    keeps the compiled function so repeated calls skip trace/lower/compile.
    """
    import jax
    from jax.experimental.shard_map import shard_map
    from jax.sharding import Mesh, PartitionSpec

    from concourse import bass2jax

    bass2jax.install_neuronx_cc_hook()
    partition_name = nc.partition_id_tensor.name if nc.partition_id_tensor else None

    in_names: list[str] = []
    out_names: list[str] = []
    out_avals = []
    zero_shapes = []
    for alloc in nc.m.functions[0].allocations:
        if not isinstance(alloc, mybir.MemoryLocationSet):
            continue
        name = alloc.memorylocations[0].name
        if alloc.kind == "ExternalInput":
            if name != partition_name:
                in_names.append(name)
        elif alloc.kind == "ExternalOutput":
            shape = tuple(alloc.tensor_shape)
            dtype = mybir.dt.np(alloc.dtype)
            out_names.append(name)
            out_avals.append(jax.core.ShapedArray(shape, dtype))
            zero_shapes.append((shape, dtype))

    n_params = len(in_names)
    n_outs = len(out_avals)
    all_in = tuple(in_names + out_names + ([partition_name] if partition_name else []))
    donate = tuple(range(n_params, n_params + n_outs))

    def _body(*args):
        operands = list(args)
        if partition_name is not None:
            operands.append(bass2jax.partition_id_tensor())
        outs = bass2jax._bass_exec_p.bind(
            *operands,
            out_avals=tuple(out_avals),
            in_names=all_in,
            out_names=tuple(out_names),
            lowering_input_output_aliases=(),
            sim_require_finite=True,
            sim_require_nnan=True,
            nc=nc,
        )
        return tuple(outs)

    devices = jax.devices()[:n_cores]
    assert len(devices) == n_cores, (len(jax.devices()), n_cores)
    mesh = Mesh(np.asarray(devices), ("core",))
    in_specs = (PartitionSpec("core"),) * (n_params + n_outs)
    out_specs = (PartitionSpec("core"),) * n_outs
    fn = jax.jit(
        shard_map(_body, mesh=mesh, in_specs=in_specs, out_specs=out_specs,
                  check_rep=False),
        donate_argnums=donate,
        keep_unused=True,
    )

    def run(in_maps):
        concat_in = [
            np.concatenate([np.asarray(m[name]) for m in in_maps], axis=0)
            for name in in_names
        ]
        concat_zeros = [
            np.zeros((n_cores * s[0], *s[1:]), d) for (s, d) in zero_shapes
        ]
        out_arrs = fn(*concat_in, *concat_zeros)
        jax.block_until_ready(out_arrs)
        return [
            {
                name: np.asarray(out_arrs[i]).reshape(n_cores, *out_avals[i].shape)[c]
                for i, name in enumerate(out_names)
            }
            for c in range(n_cores)
        ]

    return run


_RUNNER_CACHE: dict[tuple, object] = {}


def get_runner(t_steps: int = T, reps: int = 1, nch: int = NCH,
               use_gps: bool = USE_GPS, wbufs: int = WBUFS):
    key = (t_steps, reps, nch, use_gps, wbufs)
    if key not in _RUNNER_CACHE:
        _RUNNER_CACHE[key] = make_runner(
            get_nc(t_steps, reps=reps, nch=nch, use_gps=use_gps, wbufs=wbufs)
        )
    return _RUNNER_CACHE[key]


def run_cores(in_maps, t_steps: int = T, reps: int = 1, nch: int = NCH):
    run = get_runner(t_steps, reps=reps, nch=nch)
    res = run(in_maps)
    out = np.concatenate([r["loT"].T for r in res], axis=0)
    return out.astype(np.float32), res


def kernel(x, emb, W_ih, W_hh, b_ih, b_hh, W_fc, b_fc):
    in_maps = make_in_maps(x, emb, W_ih, W_hh, b_ih, b_hh, W_fc, b_fc)
    out, _ = run_cores(in_maps)
    return out


# revision 3
# speedup vs baseline: 1.1437x; 1.1437x over previous
"""GRU policy kernel for Trainium2 (8 NeuronCores, data-parallel over batch).

Problem: nn_GRUPolicy — B=2048, T=512, V=4, E=64, H=128.

  xe = emb[x]                          # [B,T,E]
  gi = xe @ W_ih.T + b_ih              # [B,T,3H]
  scan over t: GRU cell (PyTorch gate order r,z,n)
  logits = h_T @ W_fc.T + b_fc         # [B,V]

Key facts exploited:
  * V=4 so the input-side projection collapses into a [4, 3H] lookup
    table giTab = emb @ W_ih.T + b_ih (+ b_hh r/z folded in); per step it
    is realized on-device as a K=4 one-hot matmul accumulated straight
    into the same PSUM region as the recurrence matmul.
  * The one-hot itself is built ON DEVICE from uint8 token indices
    (1 byte/token over the slow axon link instead of 8 f16 one-hot
    bytes): a broadcast DMA replicates the index row across V=4
    partitions, then one DVE is_equal against a per-partition iota
    column yields the [V, chunk*BS] one-hot, split into pieces that
    interleave with the previous chunk's recurrence steps so the DVE
    never stalls the chain.
  * Everything is kept transposed ([H, batch] on 128 partitions) so the
    recurrence never needs a transpose.
  * h' = (1-z)*n + z*h  ==  z*h - (z-1)*n  -> one GPSIMD mult (p=z*h,
    off the critical path), one fused scalar_tensor_tensor
    (q=(z-1)*n) and one subtract.
  * b_hh_n rides for free inside the fused u = (ghn + b_hh_n) * r.

Sharding: batch 2048 -> 8 cores x 256; each core runs 2 independent
128-column chains, emitted interleaved by op-kind, so the serial
per-step dependency chain of one chain overlaps with engine work of
the other.

Host side: the jitted shard_map executable is compiled ONCE per process
and cached — repeated kernel() calls only pay input upload + execute.
"""

import sys

import numpy as np

for _p in ("/opt/trn_rl_repo",):
    if _p not in sys.path:
        sys.path.insert(0, _p)

from concourse import bacc, bass, mybir, tile  # noqa: E402

F16 = mybir.dt.float16
F32 = mybir.dt.float32
U8 = mybir.dt.uint8
I32 = mybir.dt.int32
AF = mybir.ActivationFunctionType
OP = mybir.AluOpType

B, T, V, E, H = 2048, 512, 4, 64, 128
N_CORES = 8
BS = B // N_CORES          # 256 batch rows per core
NCH = 2                    # independent chains per core
USE_GPS = True             # p = z*h on GPSIMD
WBUFS = 3                  # work pool depth
W = BS // NCH              # 128 batch columns per chain
CHUNK = 64                 # time steps per one-hot chunk
CMP_SPLIT = 16             # one-hot compare pieces per chunk


def build_nc(t_steps: int = T, dump_h: bool = False, reps: int = 1,
             nch: int = NCH, use_gps: bool = USE_GPS,
             wbufs: int = WBUFS) -> bass.Bass:
    nc = bacc.Bacc(None)

    xi_d = nc.dram_tensor("xi", [1, t_steps * BS], U8, kind="ExternalInput")
    wt_d = nc.dram_tensor("WT", [H, 3 * H], F16, kind="ExternalInput")
    gi_d = nc.dram_tensor("giT", [V, 3 * H], F16, kind="ExternalInput")
    wf_d = nc.dram_tensor("WfcT", [H, V], F16, kind="ExternalInput")
    bf_d = nc.dram_tensor("bfc", [V, 1], F32, kind="ExternalInput")
    bhn_d = nc.dram_tensor("bhn", [H, 1], F32, kind="ExternalInput")
    lo_d = nc.dram_tensor("loT", [V, BS], F32, kind="ExternalOutput")
    h_d = (
        nc.dram_tensor("hT", [H, BS], F32, kind="ExternalOutput")
        if dump_h
        else None
    )

    W = BS // nch
    chunk = min(CHUNK, t_steps)
    n_chunks = max(1, t_steps // chunk)
    ccols = chunk * BS                 # one-hot columns per chunk
    pieces = min(CMP_SPLIT, chunk)     # compare pieces per chunk
    pcols = ccols // pieces
    slot = chunk // pieces             # steps between compare pieces

    with tile.TileContext(nc) as tc:
        with (
            tc.tile_pool(name="const", bufs=1) as constp,
            tc.tile_pool(name="state", bufs=1) as statep,
            tc.tile_pool(name="xbp", bufs=2) as xbp,
            tc.tile_pool(name="ohp", bufs=2) as ohp,
            tc.tile_pool(name="work", bufs=wbufs) as workp,
            tc.tile_pool(name="psAB", bufs=2, space="PSUM") as psab,
            tc.tile_pool(name="psNG", bufs=2, space="PSUM") as psng,
        ):
            wt = constp.tile([H, 3 * H], F16, tag="wt")
            nc.sync.dma_start(wt[:], wt_d[:])
            gi = constp.tile([V, 3 * H], F16, tag="gi")
            nc.sync.dma_start(gi[:], gi_d[:])
            wf = constp.tile([H, V], F16, tag="wf")
            nc.sync.dma_start(wf[:], wf_d[:])
            bf = constp.tile([V, 1], F32, tag="bf")
            nc.sync.dma_start(bf[:], bf_d[:])
            bhn = constp.tile([H, 1], F32, tag="bhn")
            nc.sync.dma_start(bhn[:], bhn_d[:])
            lo = constp.tile([V, BS], F32, tag="lo")

            iv32 = constp.tile([V, 1], I32, tag="iv32")
            nc.gpsimd.iota(iv32[:], pattern=[[0, 1]], base=0, channel_multiplier=1)
            iv = constp.tile([V, 1], F32, tag="iv")
            nc.vector.tensor_copy(iv[:], iv32[:])

            h = []
            for c in range(nch):
                hc = statep.tile([H, W], F16, tag=f"h{c}")
                nc.vector.memset(hc[:], 0.0)
                h.append(hc)

            # flat chunk schedule across reps; one-hot build for element
            # i+1 is emitted interleaved with the steps of element i.
            seq = [ck for _rep in range(reps) for ck in range(n_chunks)]

            def emit_xb(ck):
                xb = xbp.tile([V, ccols], U8, tag="xb")
                nc.sync.dma_start(
                    xb[:],
                    xi_d[:, ck * ccols : (ck + 1) * ccols].broadcast_to([V, ccols]),
                )
                return xb

            def emit_cmp(oh_t, xb, piece):
                sl = slice(piece * pcols, (piece + 1) * pcols)
                nc.vector.tensor_scalar(
                    oh_t[:, sl], xb[:, sl], iv[:, 0:1], None, op0=OP.is_equal
                )

            # prologue: chunk 0 fully prepared
            xb_cur = emit_xb(seq[0])
            oh_cur = ohp.tile([V, ccols], F16, tag="oh")
            for pc in range(pieces):
                emit_cmp(oh_cur, xb_cur, pc)

            for i, ck in enumerate(seq):
                oh_t = oh_cur
                xb_nxt = oh_nxt = None
                for tl in range(chunk):
                    # pipelined one-hot build for the next chunk
                    if i + 1 < len(seq):
                        if tl == 0:
                            xb_nxt = emit_xb(seq[i + 1])
                            oh_nxt = ohp.tile([V, ccols], F16, tag="oh")
                        if tl % slot == slot - 1:
                            emit_cmp(oh_nxt, xb_nxt, tl // slot)

                    abs_, ngs, rzs, us, ns_, ps = {}, {}, {}, {}, {}, {}
                    for c in range(nch):
                        ohs = oh_t[:, tl * BS + c * W : tl * BS + (c + 1) * W]
                        ab = psab.tile([H, 2 * W], F32, tag=f"ab{c}", name=f"ab{c}")
                        ng = psng.tile([H, 3 * W], F32, tag=f"ng{c}", name=f"ng{c}")
                        abs_[c], ngs[c] = ab, ng

                        # a = gi_r(x_t) + W_r h   (both biases folded into gi)
                        nc.tensor.matmul(
                            ab[:, 0:W], gi[:, 0:H], ohs, start=True, stop=False
                        )
                        nc.tensor.matmul(
                            ab[:, 0:W], wt[:, 0:H], h[c][:], start=False, stop=True
                        )
                        # b = gi_z(x_t) + W_z h
                        nc.tensor.matmul(
                            ab[:, W : 2 * W],
                            gi[:, H : 2 * H],
                            ohs,
                            start=True,
                            stop=False,
                        )
                        nc.tensor.matmul(
                            ab[:, W : 2 * W],
                            wt[:, H : 2 * H],
                            h[c][:],
                            start=False,
                            stop=True,
                        )
                        # ghn = W_n h ; gin = gi_n(x_t)   (kept separate)
                        nc.tensor.matmul(
                            ng[:, 0:W], wt[:, 2 * H : 3 * H], h[c][:],
                            start=True, stop=True,
                        )
                        nc.tensor.matmul(
                            ng[:, W : 2 * W], gi[:, 2 * H : 3 * H], ohs,
                            start=True, stop=True,
                        )

                    for c in range(nch):
                        # r|z = sigmoid(a|b) in one ACT op
                        rz = workp.tile([H, 2 * W], F16, tag=f"rz{c}", name=f"rz{c}")
                        nc.scalar.activation(rz[:], abs_[c][:], AF.Sigmoid)
                        rzs[c] = rz
                    for c in range(nch):
                        # u = r * (ghn + b_hh_n) ; n-input c = u + gin (PSUM)
                        u = workp.tile([H, W], F16, tag=f"u{c}", name=f"u{c}")
                        nc.vector.scalar_tensor_tensor(
                            u[:], ngs[c][:, 0:W], bhn[:], rzs[c][:, 0:W],
                            op0=OP.add, op1=OP.mult,
                        )
                        us[c] = u
                        # p = z*h off the critical path
                        p_t = workp.tile([H, W], F16, tag=f"p{c}", name=f"p{c}")
                        peng = nc.gpsimd if use_gps else nc.vector
                        peng.tensor_mul(p_t[:], rzs[c][:, W : 2 * W], h[c][:])
                        ps[c] = p_t
                    for c in range(nch):
                        nc.vector.tensor_add(
                            ngs[c][:, 2 * W : 3 * W], us[c][:], ngs[c][:, W : 2 * W]
                        )
                    for c in range(nch):
                        n_t = workp.tile([H, W], F16, tag=f"n{c}", name=f"n{c}")
                        nc.scalar.activation(n_t[:], ngs[c][:, 2 * W : 3 * W], AF.Tanh)
                        ns_[c] = n_t
                    for c in range(nch):
                        q_t = workp.tile([H, W], F16, tag=f"q{c}", name=f"q{c}")
                        nc.vector.scalar_tensor_tensor(
                            q_t[:], rzs[c][:, W : 2 * W], 1.0, ns_[c][:],
                            op0=OP.subtract, op1=OP.mult,
                        )
                        nc.vector.tensor_sub(h[c][:], ps[c][:], q_t[:])

                xb_cur, oh_cur = xb_nxt, oh_nxt

            if h_d is not None:
                hd = constp.tile([H, BS], F32, tag="hd")
                for c in range(nch):
                    nc.vector.tensor_copy(hd[:, c * W : (c + 1) * W], h[c][:])
                nc.sync.dma_start(h_d[:], hd[:])

            # logits.T = W_fc @ h + b_fc
            for c in range(nch):
                lg = psab.tile([V, W], F32, tag="ab0")
                nc.tensor.matmul(lg[:], wf[:], h[c][:], start=True, stop=True)
                nc.scalar.activation(
                    lo[:, c * W : (c + 1) * W], lg[:], AF.Identity, bias=bf[:]
                )
            nc.sync.dma_start(lo_d[:], lo[:])

    nc.finalize()
    return nc


_NC_CACHE: dict[tuple, bass.Bass] = {}


def get_nc(t_steps: int = T, reps: int = 1, nch: int = NCH,
           use_gps: bool = USE_GPS, wbufs: int = WBUFS) -> bass.Bass:
    key = (t_steps, reps, nch, use_gps, wbufs)
    if key not in _NC_CACHE:
        _NC_CACHE[key] = build_nc(t_steps, reps=reps, nch=nch,
                                  use_gps=use_gps, wbufs=wbufs)
    return _NC_CACHE[key]


def make_in_maps(x, emb, W_ih, W_hh, b_ih, b_hh, W_fc, b_fc, t_steps: int = T):
    x = np.asarray(x)
    emb = np.asarray(emb, dtype=np.float32)
    W_ih = np.asarray(W_ih, dtype=np.float32)
    W_hh = np.asarray(W_hh, dtype=np.float32)
    b_ih = np.asarray(b_ih, dtype=np.float32)
    b_hh = np.asarray(b_hh, dtype=np.float32)
    W_fc = np.asarray(W_fc, dtype=np.float32)
    b_fc = np.asarray(b_fc, dtype=np.float32)

    # Fold b_ih (all gates) + b_hh (r,z only) into the gi lookup table.
    # b_hh_n must stay inside the reset product: n = tanh(gi_n + r*(W_n h + b_hh_n))
    bias = b_ih.copy()
    bias[: 2 * H] += b_hh[: 2 * H]
    gi_tab = (emb @ W_ih.T + bias).astype(np.float16)  # [V, 3H]
    wt = np.ascontiguousarray(W_hh.T).astype(np.float16)      # [H, 3H]
    wfc = np.ascontiguousarray(W_fc.T).astype(np.float16)     # [H, V]
    bfc = b_fc.reshape(V, 1).astype(np.float32)
    bhn = b_hh[2 * H :].reshape(H, 1).astype(np.float32)

    xiT = np.ascontiguousarray(x[:, :t_steps].T.astype(np.uint8))  # [t, B]
    in_maps = []
    for c in range(N_CORES):
        xi = xiT[:, c * BS : (c + 1) * BS].reshape(1, t_steps * BS)
        in_maps.append(
            {
                "xi": np.ascontiguousarray(xi),
                "WT": wt,
                "giT": gi_tab,
                "WfcT": wfc,
                "bfc": bfc,
                "bhn": bhn,
            }
        )
    return in_maps


def make_runner(nc, n_cores: int = N_CORES):
    """Compile nc into a reusable jitted shard_map executable.

    Mirrors concourse.bass2jax.run_bass_via_pjrt's multi-core path, but
    keeps the compiled function so repeated calls skip trace/lower/compile.
    """
    import jax
    from jax.experimental.shard_map import shard_map
    from jax.sharding import Mesh, PartitionSpec

    from concourse import bass2jax

    bass2jax.install_neuronx_cc_hook()
    partition_name = nc.partition_id_tensor.name if nc.partition_id_tensor else None

    in_names: list[str] = []
    out_names: list[str] = []
    out_avals = []
    zero_shapes = []
    for alloc in nc.m.functions[0].allocations:
        if not isinstance(alloc, mybir.MemoryLocationSet):
            continue
        name = alloc.memorylocations[0].name
        if alloc.kind == "ExternalInput":
            if name != partition_name:
                in_names.append(name)
        elif alloc.kind == "ExternalOutput":
            shape = tuple(alloc.tensor_shape)
            dtype = mybir.dt.np(alloc.dtype)
            out_names.append(name)
            out_avals.append(jax.core.ShapedArray(shape, dtype))
            zero_shapes.append((shape, dtype))

    n_params = len(in_names)
    n_outs = len(out_avals)
    all_in = tuple(in_names + out_names + ([partition_name] if partition_name else []))
    donate = tuple(range(n_params, n_params + n_outs))

    def _body(*args):
        operands = list(args)
        if partition_name is not None:
            operands.append(bass2jax.partition_id_tensor())
        outs = bass2jax._bass_exec_p.bind(
            *operands,
            out_avals=tuple(out_avals),
            in_names=all_in,
            out_names=tuple(out_names),
            lowering_input_output_aliases=(),
            sim_require_finite=True,
            sim_require_nnan=True,
            nc=nc,
        )
        return tuple(outs)

    devices = jax.devices()[:n_cores]
    assert len(devices) == n_cores, (len(jax.devices()), n_cores)
    mesh = Mesh(np.asarray(devices), ("core",))
    in_specs = (PartitionSpec("core"),) * (n_params + n_outs)
    out_specs = (PartitionSpec("core"),) * n_outs
    fn = jax.jit(
        shard_map(_body, mesh=mesh, in_specs=in_specs, out_specs=out_specs,
                  check_rep=False),
        donate_argnums=donate,
        keep_unused=True,
    )

    def run(in_maps):
        concat_in = [
            np.concatenate([np.asarray(m[name]) for m in in_maps], axis=0)
            for name in in_names
        ]
        concat_zeros = [
            np.zeros((n_cores * s[0], *s[1:]), d) for (s, d) in zero_shapes
        ]
        out_arrs = fn(*concat_in, *concat_zeros)
        # No block_until_ready: np.asarray both waits and fetches in one
        # axon roundtrip; an explicit block would cost a second one.
        fetched = [np.asarray(a) for a in out_arrs]
        return [
            {
                name: fetched[i].reshape(n_cores, *out_avals[i].shape)[c]
                for i, name in enumerate(out_names)
            }
            for c in range(n_cores)
        ]

    return run


_RUNNER_CACHE: dict[tuple, object] = {}


def get_runner(t_steps: int = T, reps: int = 1, nch: int = NCH,
               use_gps: bool = USE_GPS, wbufs: int = WBUFS):
    key = (t_steps, reps, nch, use_gps, wbufs)
    if key not in _RUNNER_CACHE:
        _RUNNER_CACHE[key] = make_runner(
            get_nc(t_steps, reps=reps, nch=nch, use_gps=use_gps, wbufs=wbufs)
        )
    return _RUNNER_CACHE[key]


def run_cores(in_maps, t_steps: int = T, reps: int = 1, nch: int = NCH):
    run = get_runner(t_steps, reps=reps, nch=nch)
    res = run(in_maps)
    out = np.concatenate([r["loT"].T for r in res], axis=0)
    return out.astype(np.float32), res


def kernel(x, emb, W_ih, W_hh, b_ih, b_hh, W_fc, b_fc):
    in_maps = make_in_maps(x, emb, W_ih, W_hh, b_ih, b_hh, W_fc, b_fc)
    out, _ = run_cores(in_maps)
    return out


# revision 5
# speedup vs baseline: 1.1683x; 1.0215x over previous
"""GRU policy kernel for Trainium2 (8 NeuronCores, data-parallel over batch).

Problem: nn_GRUPolicy — B=2048, T=512, V=4, E=64, H=128.

  xe = emb[x]                          # [B,T,E]
  gi = xe @ W_ih.T + b_ih              # [B,T,3H]
  scan over t: GRU cell (PyTorch gate order r,z,n)
  logits = h_T @ W_fc.T + b_fc         # [B,V]

Key facts exploited:
  * V=4 so the input-side projection collapses into a [4, 3H] lookup
    table giTab = emb @ W_ih.T + b_ih (+ b_hh r/z folded in); per step it
    is realized on-device as a K=4 one-hot matmul accumulated straight
    into the same PSUM region as the recurrence matmul.
  * The one-hot itself is built ON DEVICE from uint8 token indices
    (1 byte/token over the slow axon link instead of 8 f16 one-hot
    bytes): a broadcast DMA replicates the index row across V=4
    partitions, then one DVE is_equal against a per-partition iota
    column yields the [V, chunk*BS] one-hot, split into pieces that
    interleave with the previous chunk's recurrence steps so the DVE
    never stalls the chain.
  * Everything is kept transposed ([H, batch] on 128 partitions) so the
    recurrence never needs a transpose.
  * h' = (1-z)*n + z*h  ==  z*h - (z-1)*n  -> one GPSIMD mult (p=z*h,
    off the critical path), one fused scalar_tensor_tensor
    (q=(z-1)*n) and one subtract.
  * b_hh_n rides for free inside the fused u = (ghn + b_hh_n) * r.

Sharding: batch 2048 -> 8 cores x 256; each core runs 2 independent
128-column chains, emitted interleaved by op-kind, so the serial
per-step dependency chain of one chain overlaps with engine work of
the other.

Host side: the jitted shard_map executable is compiled ONCE per process
and cached — repeated kernel() calls only pay input upload + execute.
"""

import sys

import numpy as np

for _p in ("/opt/trn_rl_repo",):
    if _p not in sys.path:
        sys.path.insert(0, _p)

from concourse import bacc, bass, mybir, tile  # noqa: E402

F16 = mybir.dt.float16
F32 = mybir.dt.float32
U8 = mybir.dt.uint8
I32 = mybir.dt.int32
AF = mybir.ActivationFunctionType
OP = mybir.AluOpType

B, T, V, E, H = 2048, 512, 4, 64, 128
N_CORES = 8
BS = B // N_CORES          # 256 batch rows per core
NCH = 2                    # independent chains per core
USE_GPS = True             # p = z*h on GPSIMD
WBUFS = 3                  # work pool depth
W = BS // NCH              # 128 batch columns per chain
CHUNK = 64                 # time steps per one-hot chunk
CMP_SPLIT = 16             # one-hot compare pieces per chunk


def build_nc(t_steps: int = T, dump_h: bool = False, reps: int = 1,
             nch: int = NCH, use_gps: bool = USE_GPS,
             wbufs: int = WBUFS) -> bass.Bass:
    nc = bacc.Bacc(None)

    xi_d = nc.dram_tensor("xi", [1, t_steps * BS], U8, kind="ExternalInput")
    wt_d = nc.dram_tensor("WT", [H, 3 * H], F16, kind="ExternalInput")
    gi_d = nc.dram_tensor("giT", [V, 3 * H], F16, kind="ExternalInput")
    wf_d = nc.dram_tensor("WfcT", [H, V], F16, kind="ExternalInput")
    bf_d = nc.dram_tensor("bfc", [V, 1], F32, kind="ExternalInput")
    bhn_d = nc.dram_tensor("bhn", [H, 1], F32, kind="ExternalInput")
    lo_d = nc.dram_tensor("loT", [V, BS], F32, kind="ExternalOutput")
    h_d = (
        nc.dram_tensor("hT", [H, BS], F32, kind="ExternalOutput")
        if dump_h
        else None
    )

    W = BS // nch
    chunk = min(CHUNK, t_steps)
    n_chunks = max(1, t_steps // chunk)
    ccols = chunk * BS                 # one-hot columns per chunk
    pieces = min(CMP_SPLIT, chunk)     # compare pieces per chunk
    pcols = ccols // pieces
    slot = chunk // pieces             # steps between compare pieces

    with tile.TileContext(nc) as tc:
        with (
            tc.tile_pool(name="const", bufs=1) as constp,
            tc.tile_pool(name="state", bufs=1) as statep,
            tc.tile_pool(name="xbp", bufs=2) as xbp,
            tc.tile_pool(name="ohp", bufs=2) as ohp,
            tc.tile_pool(name="work", bufs=wbufs) as workp,
            tc.tile_pool(name="psAB", bufs=2, space="PSUM") as psab,
            tc.tile_pool(name="psNG", bufs=2, space="PSUM") as psng,
        ):
            wt = constp.tile([H, 3 * H], F16, tag="wt")
            nc.sync.dma_start(wt[:], wt_d[:])
            gi = constp.tile([V, 3 * H], F16, tag="gi")
            nc.sync.dma_start(gi[:], gi_d[:])
            wf = constp.tile([H, V], F16, tag="wf")
            nc.sync.dma_start(wf[:], wf_d[:])
            bf = constp.tile([V, 1], F32, tag="bf")
            nc.sync.dma_start(bf[:], bf_d[:])
            bhn = constp.tile([H, 1], F32, tag="bhn")
            nc.sync.dma_start(bhn[:], bhn_d[:])
            lo = constp.tile([V, BS], F32, tag="lo")

            iv32 = constp.tile([V, 1], I32, tag="iv32")
            nc.gpsimd.iota(iv32[:], pattern=[[0, 1]], base=0, channel_multiplier=1)
            iv = constp.tile([V, 1], F32, tag="iv")
            nc.vector.tensor_copy(iv[:], iv32[:])

            h = []
            for c in range(nch):
                hc = statep.tile([H, W], F16, tag=f"h{c}")
                nc.vector.memset(hc[:], 0.0)
                h.append(hc)

            # flat chunk schedule across reps; one-hot build for element
            # i+1 is emitted interleaved with the steps of element i.
            seq = [ck for _rep in range(reps) for ck in range(n_chunks)]

            def emit_xb(ck):
                xb = xbp.tile([V, ccols], U8, tag="xb")
                nc.sync.dma_start(
                    xb[:],
                    xi_d[:, ck * ccols : (ck + 1) * ccols].broadcast_to([V, ccols]),
                )
                return xb

            def emit_cmp(oh_t, xb, piece):
                sl = slice(piece * pcols, (piece + 1) * pcols)
                nc.vector.tensor_scalar(
                    oh_t[:, sl], xb[:, sl], iv[:, 0:1], None, op0=OP.is_equal
                )

            # prologue: chunk 0 fully prepared
            xb_cur = emit_xb(seq[0])
            oh_cur = ohp.tile([V, ccols], F16, tag="oh")
            for pc in range(pieces):
                emit_cmp(oh_cur, xb_cur, pc)

            for i, ck in enumerate(seq):
                oh_t = oh_cur
                xb_nxt = oh_nxt = None
                for tl in range(chunk):
                    # pipelined one-hot build for the next chunk
                    if i + 1 < len(seq):
                        if tl == 0:
                            xb_nxt = emit_xb(seq[i + 1])
                            oh_nxt = ohp.tile([V, ccols], F16, tag="oh")
                        if tl % slot == slot - 1:
                            emit_cmp(oh_nxt, xb_nxt, tl // slot)

                    abs_, ngs, rzs, us, ns_, ps = {}, {}, {}, {}, {}, {}
                    for c in range(nch):
                        ohs = oh_t[:, tl * BS + c * W : tl * BS + (c + 1) * W]
                        ab = psab.tile([H, 2 * W], F32, tag=f"ab{c}", name=f"ab{c}")
                        ng = psng.tile([H, 3 * W], F32, tag=f"ng{c}", name=f"ng{c}")
                        abs_[c], ngs[c] = ab, ng

                        # a = gi_r(x_t) + W_r h   (both biases folded into gi)
                        nc.tensor.matmul(
                            ab[:, 0:W], gi[:, 0:H], ohs, start=True, stop=False
                        )
                        nc.tensor.matmul(
                            ab[:, 0:W], wt[:, 0:H], h[c][:], start=False, stop=True
                        )
                        # b = gi_z(x_t) + W_z h
                        nc.tensor.matmul(
                            ab[:, W : 2 * W],
                            gi[:, H : 2 * H],
                            ohs,
                            start=True,
                            stop=False,
                        )
                        nc.tensor.matmul(
                            ab[:, W : 2 * W],
                            wt[:, H : 2 * H],
                            h[c][:],
                            start=False,
                            stop=True,
                        )
                        # ghn = W_n h ; gin = gi_n(x_t)   (kept separate)
                        nc.tensor.matmul(
                            ng[:, 0:W], wt[:, 2 * H : 3 * H], h[c][:],
                            start=True, stop=True,
                        )
                        nc.tensor.matmul(
                            ng[:, W : 2 * W], gi[:, 2 * H : 3 * H], ohs,
                            start=True, stop=True,
                        )

                    for c in range(nch):
                        # r|z = sigmoid(a|b) in one ACT op
                        rz = workp.tile([H, 2 * W], F16, tag=f"rz{c}", name=f"rz{c}")
                        nc.scalar.activation(rz[:], abs_[c][:], AF.Sigmoid)
                        rzs[c] = rz
                    for c in range(nch):
                        # u = r * (ghn + b_hh_n) ; n-input c = u + gin (PSUM)
                        u = workp.tile([H, W], F16, tag=f"u{c}", name=f"u{c}")
                        nc.vector.scalar_tensor_tensor(
                            u[:], ngs[c][:, 0:W], bhn[:], rzs[c][:, 0:W],
                            op0=OP.add, op1=OP.mult,
                        )
                        us[c] = u
                        # p = z*h off the critical path
                        p_t = workp.tile([H, W], F16, tag=f"p{c}", name=f"p{c}")
                        peng = nc.gpsimd if use_gps else nc.vector
                        peng.tensor_mul(p_t[:], rzs[c][:, W : 2 * W], h[c][:])
                        ps[c] = p_t
                    for c in range(nch):
                        nc.vector.tensor_add(
                            ngs[c][:, 2 * W : 3 * W], us[c][:], ngs[c][:, W : 2 * W]
                        )
                    for c in range(nch):
                        n_t = workp.tile([H, W], F16, tag=f"n{c}", name=f"n{c}")
                        nc.scalar.activation(n_t[:], ngs[c][:, 2 * W : 3 * W], AF.Tanh)
                        ns_[c] = n_t
                    for c in range(nch):
                        q_t = workp.tile([H, W], F16, tag=f"q{c}", name=f"q{c}")
                        nc.vector.scalar_tensor_tensor(
                            q_t[:], rzs[c][:, W : 2 * W], 1.0, ns_[c][:],
                            op0=OP.subtract, op1=OP.mult,
                        )
                        nc.vector.tensor_sub(h[c][:], ps[c][:], q_t[:])

                xb_cur, oh_cur = xb_nxt, oh_nxt

            if h_d is not None:
                hd = constp.tile([H, BS], F32, tag="hd")
                for c in range(nch):
                    nc.vector.tensor_copy(hd[:, c * W : (c + 1) * W], h[c][:])
                nc.sync.dma_start(h_d[:], hd[:])

            # logits.T = W_fc @ h + b_fc
            for c in range(nch):
                lg = psab.tile([V, W], F32, tag="ab0")
                nc.tensor.matmul(lg[:], wf[:], h[c][:], start=True, stop=True)
                nc.scalar.activation(
                    lo[:, c * W : (c + 1) * W], lg[:], AF.Identity, bias=bf[:]
                )
            nc.sync.dma_start(lo_d[:], lo[:])

    nc.finalize()
    return nc


_NC_CACHE: dict[tuple, bass.Bass] = {}


def get_nc(t_steps: int = T, reps: int = 1, nch: int = NCH,
           use_gps: bool = USE_GPS, wbufs: int = WBUFS) -> bass.Bass:
    key = (t_steps, reps, nch, use_gps, wbufs)
    if key not in _NC_CACHE:
        _NC_CACHE[key] = build_nc(t_steps, reps=reps, nch=nch,
                                  use_gps=use_gps, wbufs=wbufs)
    return _NC_CACHE[key]


def make_in_maps(x, emb, W_ih, W_hh, b_ih, b_hh, W_fc, b_fc, t_steps: int = T):
    x = np.asarray(x)
    emb = np.asarray(emb, dtype=np.float32)
    W_ih = np.asarray(W_ih, dtype=np.float32)
    W_hh = np.asarray(W_hh, dtype=np.float32)
    b_ih = np.asarray(b_ih, dtype=np.float32)
    b_hh = np.asarray(b_hh, dtype=np.float32)
    W_fc = np.asarray(W_fc, dtype=np.float32)
    b_fc = np.asarray(b_fc, dtype=np.float32)

    # Fold b_ih (all gates) + b_hh (r,z only) into the gi lookup table.
    # b_hh_n must stay inside the reset product: n = tanh(gi_n + r*(W_n h + b_hh_n))
    bias = b_ih.copy()
    bias[: 2 * H] += b_hh[: 2 * H]
    gi_tab = (emb @ W_ih.T + bias).astype(np.float16)  # [V, 3H]
    wt = np.ascontiguousarray(W_hh.T).astype(np.float16)      # [H, 3H]
    wfc = np.ascontiguousarray(W_fc.T).astype(np.float16)     # [H, V]
    bfc = b_fc.reshape(V, 1).astype(np.float32)
    bhn = b_hh[2 * H :].reshape(H, 1).astype(np.float32)

    xiT = np.ascontiguousarray(x[:, :t_steps].T.astype(np.uint8))  # [t, B]
    in_maps = []
    for c in range(N_CORES):
        xi = xiT[:, c * BS : (c + 1) * BS].reshape(1, t_steps * BS)
        in_maps.append(
            {
                "xi": np.ascontiguousarray(xi),
                "WT": wt,
                "giT": gi_tab,
                "WfcT": wfc,
                "bfc": bfc,
                "bhn": bhn,
            }
        )
    return in_maps


def make_runner(nc, n_cores: int = N_CORES):
    """Compile nc into a reusable jitted shard_map executable.

    Mirrors concourse.bass2jax.run_bass_via_pjrt's multi-core path, but
    keeps the compiled function so repeated calls skip trace/lower/compile.
    """
    import jax
    from jax.experimental.shard_map import shard_map
    from jax.sharding import Mesh, PartitionSpec

    from concourse import bass2jax

    bass2jax.install_neuronx_cc_hook()
    partition_name = nc.partition_id_tensor.name if nc.partition_id_tensor else None

    in_names: list[str] = []
    out_names: list[str] = []
    out_avals = []
    zero_shapes = []
    for alloc in nc.m.functions[0].allocations:
        if not isinstance(alloc, mybir.MemoryLocationSet):
            continue
        name = alloc.memorylocations[0].name
        if alloc.kind == "ExternalInput":
            if name != partition_name:
                in_names.append(name)
        elif alloc.kind == "ExternalOutput":
            shape = tuple(alloc.tensor_shape)
            dtype = mybir.dt.np(alloc.dtype)
            out_names.append(name)
            out_avals.append(jax.core.ShapedArray(shape, dtype))
            zero_shapes.append((shape, dtype))

    n_params = len(in_names)
    n_outs = len(out_avals)
    all_in = tuple(in_names + out_names + ([partition_name] if partition_name else []))
    donate = tuple(range(n_params, n_params + n_outs))

    def _body(*args):
        operands = list(args)
        if partition_name is not None:
            operands.append(bass2jax.partition_id_tensor())
        outs = bass2jax._bass_exec_p.bind(
            *operands,
            out_avals=tuple(out_avals),
            in_names=all_in,
            out_names=tuple(out_names),
            lowering_input_output_aliases=(),
            sim_require_finite=True,
            sim_require_nnan=True,
            nc=nc,
        )
        return tuple(outs)

    devices = jax.devices()[:n_cores]
    assert len(devices) == n_cores, (len(jax.devices()), n_cores)
    mesh = Mesh(np.asarray(devices), ("core",))
    in_specs = (PartitionSpec("core"),) * (n_params + n_outs)
    out_specs = (PartitionSpec("core"),) * n_outs
    fn = jax.jit(
        shard_map(_body, mesh=mesh, in_specs=in_specs, out_specs=out_specs,
                  check_rep=False),
        donate_argnums=donate,
        keep_unused=True,
    )

    def run_concat(concat_map):
        concat_in = [concat_map[name] for name in in_names]
        concat_zeros = [
            np.zeros((n_cores * s[0], *s[1:]), d) for (s, d) in zero_shapes
        ]
        out_arrs = fn(*concat_in, *concat_zeros)
        # No block_until_ready: np.asarray both waits and fetches in one
        # axon roundtrip; an explicit block would cost a second one.
        fetched = [np.asarray(a) for a in out_arrs]
        return [
            {
                name: fetched[i].reshape(n_cores, *out_avals[i].shape)[c]
                for i, name in enumerate(out_names)
            }
            for c in range(n_cores)
        ]

    def run(in_maps):
        concat_map = {
            name: np.concatenate([np.asarray(m[name]) for m in in_maps], axis=0)
            for name in in_names
        }
        return run_concat(concat_map)

    run.run_concat = run_concat
    return run


_RUNNER_CACHE: dict[tuple, object] = {}


def get_runner(t_steps: int = T, reps: int = 1, nch: int = NCH,
               use_gps: bool = USE_GPS, wbufs: int = WBUFS):
    key = (t_steps, reps, nch, use_gps, wbufs)
    if key not in _RUNNER_CACHE:
        _RUNNER_CACHE[key] = make_runner(
            get_nc(t_steps, reps=reps, nch=nch, use_gps=use_gps, wbufs=wbufs)
        )
    return _RUNNER_CACHE[key]


def run_cores(in_maps, t_steps: int = T, reps: int = 1, nch: int = NCH):
    run = get_runner(t_steps, reps=reps, nch=nch)
    res = run(in_maps)
    out = np.concatenate([r["loT"].T for r in res], axis=0)
    return out.astype(np.float32), res


def make_concat_inputs(x, emb, W_ih, W_hh, b_ih, b_hh, W_fc, b_fc,
                       t_steps: int = T):
    """Pre-sharded inputs as single [8*d0, ...] arrays (runner fast path)."""
    x = np.asarray(x)
    emb = np.asarray(emb, dtype=np.float32)
    W_ih = np.asarray(W_ih, dtype=np.float32)
    W_hh = np.asarray(W_hh, dtype=np.float32)
    b_ih = np.asarray(b_ih, dtype=np.float32)
    b_hh = np.asarray(b_hh, dtype=np.float32)
    W_fc = np.asarray(W_fc, dtype=np.float32)
    b_fc = np.asarray(b_fc, dtype=np.float32)

    bias = b_ih.copy()
    bias[: 2 * H] += b_hh[: 2 * H]
    gi_tab = (emb @ W_ih.T + bias).astype(np.float16)  # [V, 3H]
    wt = np.ascontiguousarray(W_hh.T).astype(np.float16)      # [H, 3H]
    wfc = np.ascontiguousarray(W_fc.T).astype(np.float16)     # [H, V]
    bfc = b_fc.reshape(V, 1).astype(np.float32)
    bhn = b_hh[2 * H :].reshape(H, 1).astype(np.float32)

    # per-core xi rows in one vectorized pass: [8, t*BS]
    xiT = x[:, :t_steps].astype(np.uint8).T           # [t, B]
    xi = np.ascontiguousarray(
        xiT.reshape(t_steps, N_CORES, BS).transpose(1, 0, 2)
    ).reshape(N_CORES, t_steps * BS)

    def rep(a):
        return np.ascontiguousarray(
            np.broadcast_to(a, (N_CORES, *a.shape))
        ).reshape(N_CORES * a.shape[0], *a.shape[1:])

    return {
        "xi": xi,
        "WT": rep(wt),
        "giT": rep(gi_tab),
        "WfcT": rep(wfc),
        "bfc": rep(bfc),
        "bhn": rep(bhn),
    }


def kernel(x, emb, W_ih, W_hh, b_ih, b_hh, W_fc, b_fc):
    concat_map = make_concat_inputs(x, emb, W_ih, W_hh, b_ih, b_hh, W_fc, b_fc)
    run = get_runner()
    res = run.run_concat(concat_map)
    out = np.concatenate([r["loT"].T for r in res], axis=0)
    return out.astype(np.float32)
